# revision 15
# baseline (speedup 1.0000x reference)
"""Trainium2 Bass kernel for CropProposals (adaptive max-pool 2x2x2 over
data-dependent crops of a [4,128,24,24,24] feature map).

v5 design
---------
Volume split + global balance: the two cores of each batch split the batch
volume along a host-chosen axis at plane m; overloaded cores then donate
plane-window piece groups of their batch to underloaded cores of OTHER
batches, which load those planes as appended "guest" slabs.  fm streams as
fp16 (rel err ~5e-4 << the 2e-2 gate).

DVE instruction economy (measured: 59ns + free_elems * 1.05ns packed /
1.70ns strided-inner):
 - pieces with <=2 dims >1 keep both remaining octant pairs in ONE reduce;
 - both regions of a proposal merge into one 8-output reduce when unclipped
   together with r<=1;
 - a per-slab rest-axis swap picks which semantic axis gets stride 1.

Streaming: 2-plane chunks; chunks 0-1 lead, the rest release once the
vector engine has dispatched into its branch (the branch IRAM fetch shares
the DMA queues).  Post-gate even chunks issue from SP, odd from the scalar
engine, so descriptor generation is not serial.  The scalar engine also
DMAs the fp16 output on a one-shot semaphore from the last reduce.
"""

import numpy as np

_B, _C, _D, _H, _W = 4, 128, 24, 24, 24
_P = 64
_NCORES = 8
_SIZE = 24
_PLANE = _SIZE * _SIZE
_CHUNK_PLANES = 2
_MAX_PLANES = 24          # per-core slab capacity (12 chunks)
_MAX_COLS = 512

# calibrated timing model (ns)
_BUB = 59.0
_E_PACK = 1.05
_E_STRIDE = 1.70
_VSTART = 11280.0
_END_PAD = 330.0


def _sem_time(ci):
    if ci < 2:
        return 11300.0 + 760.0 * ci
    return 13400.0 + 780.0 * (ci - 2)


_cache = {}


def _box_params(corners, scale):
    c = np.asarray(corners).astype(np.int64)
    p1 = np.clip(c[:, :, 0, :] // scale, 0, 21)
    p2r = c[:, :, 1, :] // scale
    p2 = np.where(p2r - p1 >= 2, p2r, p1 + 2)
    sizes = np.array([_D, _H, _W], dtype=np.int64)
    e = np.minimum(p2, sizes)
    n = e - p1
    l = (n + 1) // 2
    dlt = n // 2
    return p1, l, dlt


def _icost(free, inner_stride, inner_len):
    e = _E_PACK if (inner_stride == 1 and inner_len >= 2) else _E_STRIDE
    return _BUB + free * e


def _plan_slab(s, l, dlt, b, axis, h0, h1, flip, swap, plane_base,
               only_inside=False, skip=None):
    """Emit plan for pieces of batch `b` intersecting [h0,h1) along `axis`.

    `plane_base` offsets the slab inside the core's SBUF layout.  With
    `only_inside`, only regions fully inside [h0,h1) are planned (guest
    slabs).  `skip` is a set of (p, oa) to omit (donated elsewhere).
    Emits carry blk=(b,p,oa) (col assigned later).
    """
    rest = [a for a in range(3) if a != axis]
    n_pl = h1 - h0
    str_r0, str_r1 = (_SIZE, 1) if not swap else (1, _SIZE)
    emits = []

    def layout_u(cs, ce):
        u0, u1 = cs - h0, ce - h0
        if flip:
            u0, u1 = n_pl - u1, n_pl - u0
        return u0 + plane_base, u1 + plane_base

    for p in range(_P):
        la_f, sa, da = int(l[b, p, axis]), int(s[b, p, axis]), int(dlt[b, p, axis])
        l2, l3 = int(l[b, p, rest[0]]), int(l[b, p, rest[1]])
        s2, s3 = int(s[b, p, rest[0]]), int(s[b, p, rest[1]])
        d2, d3 = int(dlt[b, p, rest[0]]), int(dlt[b, p, rest[1]])
        roff = s2 * str_r0 + s3 * str_r1
        pieces = []
        for oa in range(2):
            if skip and (p, oa) in skip:
                continue
            st = sa + oa * da
            if only_inside:
                if st < h0 or st + la_f > h1:
                    continue
                cs, ce = st, st + la_f
            else:
                cs, ce = max(st, h0), min(st + la_f, h1)
            if cs < ce:
                pieces.append((oa, cs, ce))
        if not pieces:
            continue

        r_rest = (l2 > 1) + (l3 > 1)
        if (len(pieces) == 2
                and pieces[0][2] - pieces[0][1] == la_f
                and pieces[1][2] - pieces[1][1] == la_f
                and (la_f > 1) + r_rest <= 1):
            u0a, u1a = layout_u(pieces[0][1], pieces[0][2])
            u0b, u1b = layout_u(pieces[1][1], pieces[1][2])
            base = min(u0a, u0b)
            kept = [[da * _PLANE, 2],
                    [d2 * str_r0, 2], [d3 * str_r1, 2]]
            red = []
            if la_f > 1:
                red.append([_PLANE, la_f])
            if l2 > 1:
                red.append([str_r0, l2])
            if l3 > 1:
                red.append([str_r1, l3])
            if not red:
                red = [[1, 1]]
            free = 8 * la_f * l2 * l3
            dur = _icost(free, red[-1][0], red[-1][1])
            req = (max(u1a, u1b) - 1) // _CHUNK_PLANES
            emits.append(dict(
                kind='merge', blk=(b, p), off=base * _PLANE + roff,
                kept=kept, red=red,
                oa_first=(1 if flip else 0), req=req, dur=dur))
            continue

        for oa, cs, ce in pieces:
            la = ce - cs
            u0, u1 = layout_u(cs, ce)
            off = u0 * _PLANE + roff
            req = (u1 - 1) // _CHUNK_PLANES
            vol = la * l2 * l3
            dims_all = [[_PLANE, la], [str_r0, l2], [str_r1, l3]]
            red_full = sorted([d for d in dims_all if d[1] > 1],
                              key=lambda x: -abs(x[0]))
            if len(red_full) <= 2:
                red1 = red_full if red_full else [[1, 1]]
                dur = _icost(4 * vol, red1[-1][0], red1[-1][1])
                emits.append(dict(
                    kind='one', blk=(b, p, oa), off=off,
                    kept=[[d2 * str_r0, 2], [d3 * str_r1, 2]],
                    red=red1, req=req, dur=dur))
            else:
                dur = _icost(2 * vol, red_full[-1][0], red_full[-1][1])
                for o2 in range(2):
                    emits.append(dict(
                        kind='two', blk=(b, p, oa), o2=o2,
                        off=off + o2 * d2 * str_r0,
                        kept=[[d3 * str_r1, 2]],
                        red=red_full, req=req, dur=dur))
    return emits


def _sim_core(emits):
    t = _VSTART
    for e in sorted(emits, key=lambda e: (e['req'], e['blk'])):
        t = max(t, _sem_time(e['req'])) + e['dur']
    return t + _END_PAD


def _optimize_batch(s, l, dlt, b):
    best = None
    for axis in range(3):
        for m in range(6, 19):
            plans = {}
            for h, (h0, h1) in enumerate(((0, m), (m, _SIZE))):
                plans[h] = {}
                for f in (False, True):
                    for sw in (False, True):
                        e = _plan_slab(s, l, dlt, b, axis, h0, h1, f, sw, 0)
                        plans[h][(f, sw)] = (e, _sim_core(e))
            for (f0, sw0), (e0, t0) in plans[0].items():
                for (f1, sw1), (e1, t1) in plans[1].items():
                    worst = max(t0, t1)
                    if best is None or worst < best[0]:
                        best = (worst, axis, m, (f0, f1), (sw0, sw1), (e0, e1))
    return best


def _global_balance(s, l, dlt, batch_cfg):
    """Greedy cross-batch stealing.

    batch_cfg[b] = (axis, m, flips, swaps, emits0, emits1).
    Returns per-core dicts: own-slab spec + guest slab specs + final emits.
    """
    cores = []
    for b in range(_B):
        axis, m, flips, swaps, e0, e1 = batch_cfg[b]
        for h, (h0, h1) in enumerate(((0, m), (m, _SIZE))):
            cores.append(dict(
                b=b, axis=axis, h0=h0, h1=h1, flip=flips[h], swap=swaps[h],
                own_emits=list((e0, e1)[h]), guests=[], guest_emits=[],
                skip=set()))

    def planes_of(c):
        own = c["h1"] - c["h0"]
        return own + sum(g[2] - g[1] for g in c["guests"])

    def emits_of(c):
        return c["own_emits"] + c["guest_emits"]

    def replan_own(c):
        c["own_emits"] = _plan_slab(
            s, l, dlt, c["b"], c["axis"], c["h0"], c["h1"],
            c["flip"], c["swap"], 0, skip=c["skip"])

    fins = [_sim_core(emits_of(c)) for c in cores]
    for _round in range(12):
        order = sorted(range(_NCORES), key=lambda k: -fins[k])
        donor = order[0]
        moved = False
        dc = cores[donor]
        # candidate windows inside donor's own half
        h0, h1 = dc["h0"], dc["h1"]
        for width in (6, 4, 8, 3):
            for g0 in range(h0, h1 - width + 1):
                g1 = g0 + width
                # movable pieces: fully inside [g0,g1), not already donated
                mv = []
                for p in range(_P):
                    for oa in range(2):
                        if (p, oa) in dc["skip"]:
                            continue
                        st = int(s[dc["b"], p, dc["axis"]]
                                 + oa * dlt[dc["b"], p, dc["axis"]])
                        en = st + int(l[dc["b"], p, dc["axis"]])
                        if g0 <= st and en <= g1 and st >= h0 and en <= h1:
                            mv.append((p, oa))
                if len(mv) < 4:
                    continue
                for recip in order[::-1][:3]:
                    if recip == donor:
                        continue
                    rc = cores[recip]
                    if planes_of(rc) + width > _MAX_PLANES:
                        continue
                    base = planes_of(rc)
                    # plan guest emits (try both swaps)
                    bestg = None
                    for sw in (False, True):
                        ge = _plan_slab(
                            s, l, dlt, dc["b"], dc["axis"], g0, g1,
                            False, sw, base,
                            only_inside=True,
                            skip={(p, oa) for p in range(_P)
                                  for oa in range(2)
                                  if (p, oa) not in mv} | dc["skip"])
                        cost = sum(e['dur'] for e in ge)
                        if bestg is None or cost < bestg[1]:
                            bestg = (ge, cost, sw)
                    ge, _, gsw = bestg
                    if not ge:
                        continue
                    # try the move
                    old_skip = set(dc["skip"])
                    dc["skip"] |= set(mv)
                    old_own = dc["own_emits"]
                    replan_own(dc)
                    new_fin_d = _sim_core(emits_of(dc))
                    new_fin_r = _sim_core(emits_of(rc) + ge)
                    if max(new_fin_d, new_fin_r) < fins[donor] - 150:
                        rc["guests"].append((dc["b"], g0, g1, gsw, base,
                                             dc["axis"]))
                        rc["guest_emits"] += ge
                        fins[donor] = new_fin_d
                        fins[recip] = new_fin_r
                        moved = True
                        break
                    dc["skip"] = old_skip
                    dc["own_emits"] = old_own
                if moved:
                    break
            if moved:
                break
        if not moved:
            break

    # final col assignment per core
    for c in cores:
        cur = 0
        blkcol = {}
        for e in sorted(emits_of(c), key=lambda e: (e['req'], e['blk'])):
            if e['kind'] == 'merge':
                e['col'] = cur
                cur += 8
            elif e['kind'] == 'one':
                e['col'] = cur
                cur += 4
            else:  # two
                key = e['blk']
                if key not in blkcol:
                    blkcol[key] = cur
                    cur += 4
                e['col'] = blkcol[key] + e['o2'] * 2
        assert cur <= _MAX_COLS, cur
        c["emits"] = emits_of(c)
    return cores


def _build_program(cores):
    import concourse.bacc as bacc
    import concourse.bass as bass_mod
    import concourse.mybir as mybir
    from concourse.ap import AP
    from contextlib import ExitStack

    n_pl_max = max(c["h1"] - c["h0"]
                   + sum(g[2] - g[1] for g in c["guests"]) for c in cores)
    n_chunks = (n_pl_max + _CHUNK_PLANES - 1) // _CHUNK_PLANES
    vol_elems = n_chunks * _CHUNK_PLANES * _PLANE

    orig_memset = bass_mod.BassGpSimd.memset
    orig_barrier = bass_mod.Bass.all_engine_barrier
    bass_mod.BassGpSimd.memset = lambda self, ap, c: None
    bass_mod.Bass.all_engine_barrier = lambda self, **kw: None
    try:
        nc = bacc.Bacc("TRN2", target_bir_lowering=False, debug=False,
                       num_devices=_NCORES)
    finally:
        bass_mod.BassGpSimd.memset = orig_memset
        bass_mod.Bass.all_engine_barrier = orig_barrier

    x_in = nc.dram_tensor("fm", [_C, vol_elems], mybir.dt.float16,
                          kind="ExternalInput")
    y_out = nc.dram_tensor("out", [_C, _MAX_COLS], mybir.dt.float16,
                           kind="ExternalOutput")

    with ExitStack() as stk:
        xt = stk.enter_context(
            nc.sbuf_tensor("xt", [_C, vol_elems], mybir.dt.float16))
        yt = stk.enter_context(
            nc.sbuf_tensor("yt", [_C, _MAX_COLS], mybir.dt.float16))
        csems = [stk.enter_context(nc.semaphore(f"dma_sem{i}"))
                 for i in range(n_chunks)]
        out_sem = stk.enter_context(nc.semaphore("out_sem"))
        v_sem = stk.enter_context(nc.semaphore("v_sem"))
        ready_sem = stk.enter_context(nc.semaphore("ready_sem"))
        block = stk.enter_context(nc.Block())

        def chunk_sl(ci):
            return slice(ci * _CHUNK_PLANES * _PLANE,
                         (ci + 1) * _CHUNK_PLANES * _PLANE)

        @block.sync
        def _(sync):
            # chunks 0-1 lead; post-gate SP takes even chunks
            for ci in range(2):
                sync.dma_start(out=xt[:, chunk_sl(ci)],
                               in_=x_in[:, chunk_sl(ci)]).then_inc(csems[ci], 16)
            sync.wait_ge(ready_sem, 1)
            for ci in range(2, n_chunks, 2):
                sync.dma_start(out=xt[:, chunk_sl(ci)],
                               in_=x_in[:, chunk_sl(ci)]).then_inc(csems[ci], 16)
            sync.wait_ge(out_sem, 16)

        @block.scalar
        def _(scalar):
            scalar.wait_ge(ready_sem, 1)
            for ci in range(3, n_chunks, 2):
                scalar.dma_start(out=xt[:, chunk_sl(ci)],
                                 in_=x_in[:, chunk_sl(ci)]).then_inc(csems[ci], 16)
            scalar.wait_ge(v_sem, 1)
            scalar.dma_start(out=y_out[:], in_=yt[:]).then_inc(out_sem, 16)

        pid_holder = []

        @block.vector
        def _(vector):
            pid = vector.partition_id()
            pid_holder.append(pid)
            hint = vector.switch_hint(pid, _NCORES, "disp")
            base = xt[:]
            part_dim = list(base.ap[0])
            for k in vector.Switch(pid, _NCORES, hint=hint):
                vector.engine_nop().then_inc(ready_sem, 1)
                emits = cores[k]["emits"]
                if not emits:
                    vector.engine_nop().then_inc(v_sem, 1)
                    continue
                order = sorted(range(len(emits)),
                               key=lambda i: (emits[i]['req'], emits[i]['blk']))
                waited = 0
                for n_done, idx in enumerate(order):
                    e = emits[idx]
                    while waited <= e['req']:
                        vector.wait_ge(csems[waited], 16)
                        waited += 1
                    kept = e['kept']
                    nred = len([d for d in e['red'] if d[1] > 1])
                    axis_t = {1: mybir.AxisListType.X,
                              2: mybir.AxisListType.XY,
                              3: mybir.AxisListType.XYZ}[max(nred, 1)]
                    ap = AP(base.tensor, base.offset + e['off'],
                            [part_dim] + kept + e['red'])
                    wid = 1 << len(kept)
                    r = vector.tensor_reduce(
                        out=yt[:, e['col']:e['col'] + wid], in_=ap,
                        axis=axis_t, op=mybir.AluOpType.max)
                    if n_done == len(order) - 1:
                        r.then_inc(v_sem, 1)

        pid_sv = pid_holder[0]
        for eng in nc.engines.values():
            if eng._cached_partition_id is None:
                eng._cached_partition_id = pid_sv
        nc._cached_partition_id_multi[tuple(mybir.ALL_ENGINES)] = pid_sv

    nc.compile()
    return nc, n_chunks


def _get_program(corners, scale):
    key = (np.asarray(corners).tobytes(), int(scale))
    if key not in _cache:
        s, l, dlt = _box_params(corners, scale)
        batch_cfg = []
        for b in range(_B):
            worst, axis, m, flips, swaps, (e0, e1) = _optimize_batch(s, l, dlt, b)
            batch_cfg.append((axis, m, flips, swaps, e0, e1))
        cores = _global_balance(s, l, dlt, batch_cfg)
        nc, n_chunks = _build_program(cores)
        _cache[key] = (nc, cores, n_chunks)
    return _cache[key]


def _install_ntff_shim():
    import sys
    import types
    try:
        import antenv.axon_hooks  # noqa: F401
        return
    except ImportError:
        pass
    try:
        from trn_agent_boot.trn_boot import _ntff_profile_via_ctypes
        hook = _ntff_profile_via_ctypes("/opt/axon/libaxon_pjrt.so")
        mod = types.ModuleType("antenv.axon_hooks")
        mod._hook = hook
        mod.get_axon_ntff_profile_hook = lambda: mod._hook

        def _set(h):
            mod._hook = h

        mod.set_axon_ntff_profile_hook = _set
        sys.modules["antenv.axon_hooks"] = mod
        import antenv
        antenv.axon_hooks = mod
    except Exception:
        pass


def _slab_view(fm16, b, axis, h0, h1, flip, swap):
    vol = fm16[b]
    if axis != 0:
        rest = [a for a in range(3) if a != axis]
        vol = np.transpose(vol, (0, 1 + axis, 1 + rest[0], 1 + rest[1]))
    vol = vol[:, h0:h1]
    if flip:
        vol = vol[:, ::-1]
    if swap:
        vol = np.swapaxes(vol, 2, 3)
    return np.ascontiguousarray(vol).reshape(_C, -1)


def _run(fm, corners, scale, trace=False, trace_cores=None):
    from concourse.bass_utils import run_bass_kernel_spmd
    if trace:
        _install_ntff_shim()

    fm = np.asarray(fm, dtype=np.float32)
    scale = int(scale)
    nc, cores, n_chunks = _get_program(corners, scale)
    vol_elems = n_chunks * _CHUNK_PLANES * _PLANE

    fm16 = fm.astype(np.float16)
    in_maps = []
    for k in range(_NCORES):
        c = cores[k]
        buf = np.zeros((_C, vol_elems), dtype=np.float16)
        own = _slab_view(fm16, c["b"], c["axis"], c["h0"], c["h1"],
                         c["flip"], c["swap"])
        buf[:, :own.shape[1]] = own
        for (gb, g0, g1, gsw, gbase, gaxis) in c["guests"]:
            gv = _slab_view(fm16, gb, gaxis, g0, g1, False, gsw)
            buf[:, gbase * _PLANE: gbase * _PLANE + gv.shape[1]] = gv
        in_maps.append({"fm": buf})

    kwargs = {}
    if trace:
        kwargs.update(trace=True,
                      trace_cores=trace_cores or list(range(_NCORES)))
    res = run_bass_kernel_spmd(nc, in_maps, list(range(_NCORES)), **kwargs)

    # decode: gather contributions per (b,p,oa) block, max-combine
    axes = {}
    for c in cores:
        axes[c["b"]] = c["axis"]
    blocks = {}

    def add_block(key, arr):
        if key in blocks:
            blocks[key] = np.maximum(blocks[key], arr)
        else:
            blocks[key] = arr

    part = {}
    for k in range(_NCORES):
        y = res.results[k]["out"].astype(np.float32)
        for e in cores[k]["emits"]:
            if e['kind'] == 'merge':
                b, p = e['blk']
                for i_k, oa in enumerate((e['oa_first'], 1 - e['oa_first'])):
                    add_block((b, p, oa), y[:, e['col'] + i_k * 4:
                                            e['col'] + i_k * 4 + 4])
            elif e['kind'] == 'one':
                add_block(e['blk'], y[:, e['col']:e['col'] + 4])
            else:
                key = (k, e['blk'])
                half = y[:, e['col']:e['col'] + 2]
                if key in part:
                    other = part.pop(key)
                    blk = np.empty((_C, 4), np.float32)
                    o2 = e['o2']
                    blk[:, o2 * 2:o2 * 2 + 2] = half
                    blk[:, (1 - o2) * 2:(1 - o2) * 2 + 2] = other[1]
                    add_block(e['blk'], blk)
                else:
                    part[key] = (e['o2'], half)

    out = np.empty((_B, _P, _C, 2, 2, 2), dtype=np.float32)
    for b in range(_B):
        axis = axes[b]
        rest = [a for a in range(3) if a != axis]
        for p in range(_P):
            for oa in range(2):
                blk = blocks[(b, p, oa)]
                idx = [None, None, None]
                for o2 in range(2):
                    for o3 in range(2):
                        idx[axis] = oa
                        idx[rest[0]] = o2
                        idx[rest[1]] = o3
                        out[b, p, :, idx[0], idx[1], idx[2]] = blk[:, o2 * 2 + o3]
    return out, getattr(res, "exec_time_ns", None)


def kernel(fm, corners, scale=4):
    out, _ = _run(fm, corners, scale, trace=False)
    return out
